# revision 27
# baseline (speedup 1.0000x reference)
"""Trainium2 Bass kernel for nn_Bezier (quadratic Bezier curve rasterization).

Reference semantics: 65536 curve samples, each scatter-adds a 32x32 truncated
Gaussian patch exp(-((x-ci)^2+(y-cj)^2)/(2*sigma^2)) into a 2048x2048 grid at
block corner (bx,by) = clip(floor(2048*curve)-16, 0, 2016); output is the
mean over samples.

Device algorithm (8 NeuronCores, SPMD), v2:
  The patch is separable (outer product of two 32-vectors), so each block of
  128 consecutive samples becomes one TensorE matmul contracting over the
  samples:  window[48x48] += SX.T @ SY,  where SX[k, i] is sample k's masked
  Gaussian strip over a 48-wide x-window and SY[k, j] the y-strip.  Two
  consecutive blocks (256 samples, coordinate drift <= 16 px guaranteed by
  |B'| <= 2) share one window and accumulate in PSUM.

  Strips are built without any per-sample tables:
    exponent T[k,i] = -INV*(x'_k - c'_i)^2 expands into a rank-3 bilinear
    form, so one tiny fp16 matmul per block computes the whole [128 x 48]
    exponent tile:  lhsT = [-INV*x'^2; 2*INV*x'; 1] (device-computed from
    control_points, PE-transposed into sample-major basis tiles), rhs = the
    CONSTANT [1; c'; -INV*c'^2] since columns are recentered at each
    window's center (c'_i = (i-24)/2048 for every window).  Matmul operands
    must start at partition 0/32/64, so basis rows live in 32-partition
    bands (3 blocks per 128-wide PE transpose) and the rhs constant is
    replicated at the three bases.  Exponents for 8 blocks land in one PSUM
    bank; a single ScalarE Exp produces the fp16 Gaussian tile and one
    VectorE multiply applies the host-built exact {0, 2^-8} fp16 mask
    (whose x*y product folds in the 1/65536 normalization).

  The host only mirrors the reference's float32 index math to plan integer
  window origins and the 0/1 masks (scheduling metadata); all float curve
  values are computed on device from the control_points input.  Per-core
  I/O is a handful of resident input DMAs + 1 journal DMA (per-group HWDGE
  descriptor overhead was the previous bottleneck).  The host places the
  32 disjointly-computed per-pair windows of each core into the full grid.
"""
import os
import numpy as np
from contextlib import ExitStack

RES = 2048
STEPS = 65536
SIGMA = 0.01
W = 32
INV = np.float32(1.0 / (2.0 * SIGMA * SIGMA))   # 5000.0
NCORES = 8
SPC = STEPS // NCORES      # samples per core = 8192
NB = SPC // 128            # blocks of 128 samples per core = 64
NP_ = NB // 2              # pairs (two blocks share a window) = 32
NSG = 8                    # supergroups
BPS = NB // NSG            # blocks per supergroup = 8
WIN = 48                   # window width (32 + max drift 16)
NG = (NB + 2) // 3         # 3-block transpose groups = 22
MCOL = 176                 # mask table column offset in the f16 const tensor
FCOLS = MCOL + NSG * 768   # f16 const tensor width

LAST_RESULT = None  # BassKernelResults of the last run (for test harness)
LAST_NC = None
LAST_IN_MAPS = None
LAST_METAS = None


# ----------------------------------------------------------------- planning
def _plan(cp: np.ndarray):
    """Host planning: mirrors the reference's float32 index math exactly,
    then builds per-core window origins + fp16 mask tables."""
    p0, p1, p2 = cp[0], cp[1], cp[2]

    # exact mirror of jnp.linspace(0, 1, STEPS, dtype=float32)
    t_lin = np.empty(STEPS, np.float32)
    t_lin[: STEPS - 1] = np.arange(STEPS - 1, dtype=np.float32) / np.float32(
        STEPS - 1
    )
    t_lin[STEPS - 1] = 1.0
    t_out = np.arange(STEPS, dtype=np.float32) / np.float32(STEPS)

    a = p0[:, None] + (p1 - p0)[:, None] * t_lin
    b = p1[:, None] + (p2 - p1)[:, None] * t_lin
    curve = (a + t_out * (b - a)).astype(np.float32)          # [2, S]
    blocks = np.clip(
        np.floor(RES * curve).astype(np.int32) - W // 2, 0, RES - W
    )
    bx, by = blocks[0], blocks[1]

    # device basis tables (float32), pure functions of the step index
    U = (t_lin + t_out).astype(np.float32)
    V = (t_lin * t_out).astype(np.float32)

    # constant column basis: c'_i = (i - 24)/RES (exact in fp16), replicated
    # at partition bases 0/32/64 to satisfy matmul base alignment
    ci = (np.arange(WIN, dtype=np.float32) - 24.0) / np.float32(RES)
    cbs3 = np.zeros((128, WIN), np.float16)
    for base in (0, 32, 64):
        cbs3[base + 0] = 1.0
        cbs3[base + 1] = ci.astype(np.float16)
        cbs3[base + 2] = (-INV * ci * ci).astype(np.float16)

    ident = np.eye(128, dtype=np.float16)

    in_maps = []
    metas = []
    offs = np.arange(WIN, dtype=np.int32)[None, :]
    for c in range(NCORES):
        lo = c * SPC
        bxc = bx[lo: lo + SPC].reshape(NB, 128)
        byc = by[lo: lo + SPC].reshape(NB, 128)

        # per-pair window origins
        ox = np.minimum(bxc.reshape(NP_, 256).min(axis=1), RES - WIN)
        oy = np.minimum(byc.reshape(NP_, 256).min(axis=1), RES - WIN)
        assert (bxc.reshape(NP_, 256).max(axis=1) + W <= ox + WIN).all()
        assert (byc.reshape(NP_, 256).max(axis=1) + W <= oy + WIN).all()

        # masks: value 2^-8 inside the live 32-window (x*y product = 1/65536)
        lox = (bxc - np.repeat(ox, 2)[:, None]).astype(np.int32)  # [NB,128]
        loy = (byc - np.repeat(oy, 2)[:, None]).astype(np.int32)
        mx = ((offs[None] >= lox[:, :, None])
              & (offs[None] < lox[:, :, None] + W))
        my = ((offs[None] >= loy[:, :, None])
              & (offs[None] < loy[:, :, None] + W))
        mx = (mx.astype(np.float16) * np.float16(2.0 ** -8))
        my = (my.astype(np.float16) * np.float16(2.0 ** -8))

        # f16 const tensor: [ident | cbs3 | per-sg (mx_sg | my_sg) masks]
        fct = np.zeros((128, FCOLS), np.float16)
        fct[:, 0:128] = ident
        fct[:, 128:MCOL] = cbs3
        for sg in range(NSG):
            mb = mx[sg * BPS:(sg + 1) * BPS]          # [BPS, 128, WIN]
            yb = my[sg * BPS:(sg + 1) * BPS]
            s = MCOL + sg * 768
            fct[:, s: s + 384] = (
                mb.transpose(1, 0, 2).reshape(128, BPS * WIN)
            )
            fct[:, s + 384: s + 768] = (
                yb.transpose(1, 0, 2).reshape(128, BPS * WIN)
            )

        # window-center tables (f32, exact dyadic)
        ccx = np.repeat((ox + 24).astype(np.float32) / np.float32(RES), 2)
        ccy = np.repeat((oy + 24).astype(np.float32) / np.float32(RES), 2)

        uvc = np.zeros((128, 390), np.float32)
        uvc[:, 0:64] = U[lo: lo + SPC].reshape(NB, 128).T
        uvc[:, 64:128] = V[lo: lo + SPC].reshape(NB, 128).T
        uvc[:, 128:134] = cp.reshape(1, 6).astype(np.float32)
        uvc[:, 134:198] = np.broadcast_to(ccx, (128, NB))
        uvc[:, 198:262] = np.broadcast_to(ccy, (128, NB))
        uvc[:, 262:390] = np.eye(128, dtype=np.float32)

        in_maps.append({"uvc": uvc, "fct": fct})
        metas.append(list(zip(ox.tolist(), oy.tolist())))
    return in_maps, metas


# ------------------------------------------------------------------- device
def _build():
    import concourse.bass as bass
    import concourse.tile as tile
    from concourse import bacc, mybir

    f32 = mybir.dt.float32
    f16 = mybir.dt.float16
    Exp = mybir.ActivationFunctionType.Exp
    mult = mybir.AluOpType.mult
    add = mybir.AluOpType.add
    sub = mybir.AluOpType.subtract

    nc = bacc.Bacc(
        "TRN2", target_bir_lowering=False, debug=False, num_devices=NCORES
    )
    t_uvc = nc.dram_tensor("uvc", [128, 390], f32, kind="ExternalInput").ap()
    t_fct = nc.dram_tensor(
        "fct", [128, FCOLS], f16, kind="ExternalInput"
    ).ap()
    t_out = nc.dram_tensor(
        "out", [112, 2 * 384], f16, kind="ExternalOutput"
    ).ap()

    with tile.TileContext(nc, num_cores=NCORES) as tc, ExitStack() as ctx:
        cpool = ctx.enter_context(tc.tile_pool(name="const", bufs=1))
        sp = ctx.enter_context(tc.tile_pool(name="stream", bufs=2))
        pt = ctx.enter_context(tc.tile_pool(name="psumT", bufs=2,
                                            space="PSUM"))
        pj = ctx.enter_context(tc.tile_pool(name="psumJ", bufs=2,
                                            space="PSUM"))

        uvc = cpool.tile([128, 390], f32, tag="uvc")
        nc.sync.dma_start(uvc[:], t_uvc)
        fct = cpool.tile([128, FCOLS], f16, tag="fct")
        # first chunk carries ident+cbs3+sg0/1 masks; 3 more chunks follow
        cuts = [0, MCOL + 2 * 768, MCOL + 4 * 768, MCOL + 6 * 768, FCOLS]
        for q in range(4):
            nc.sync.dma_start(
                fct[:, cuts[q]:cuts[q + 1]], t_fct[:, cuts[q]:cuts[q + 1]]
            )

        Ut = uvc[:, 0:64]
        Vt = uvc[:, 64:128]
        cpb = uvc[:, 128:134]
        ccx = uvc[:, 134:198]
        ccy = uvc[:, 198:262]
        ident = fct[:, 0:128]
        cbs3 = fct[:, 128:MCOL]

        # curve coefficients: c1 = p1-p0, c2 = p0-2*p1+p2
        coef = cpool.tile([128, 4], f32, tag="coef")
        nc.vector.tensor_tensor(
            coef[:, 0:2], cpb[:, 2:4], cpb[:, 0:2], op=sub
        )
        nc.vector.scalar_tensor_tensor(
            coef[:, 2:4], cpb[:, 2:4], -2.0, cpb[:, 4:6], op0=mult, op1=add
        )
        nc.vector.tensor_tensor(
            coef[:, 2:4], coef[:, 2:4], cpb[:, 0:2], op=add
        )

        # basis rows packed for the PE transpose: PX4[k, g, bl, r32] where
        # (g, bl) is the 3-block transpose grouping, r32 the padded row
        PX4 = cpool.tile([128, NG, 3, 32], f16, tag="px4")
        PY4 = cpool.tile([128, NG, 3, 32], f16, tag="py4")
        nc.gpsimd.memset(PX4[:], 0.0)
        nc.gpsimd.memset(PY4[:], 0.0)

        def axis_basis(eng, c0, c1, c2, cc, P4, tag):
            t1 = cpool.tile([128, NB], f32, tag=f"t1{tag}")
            eng.tensor_scalar(t1[:], Ut, c1, None, op0=mult)
            xw = cpool.tile([128, NB], f32, tag=f"xw{tag}")
            eng.scalar_tensor_tensor(xw[:], Vt, c2, t1[:], op0=mult, op1=add)
            xp = cpool.tile([128, NB], f32, tag=f"xp{tag}")
            eng.scalar_tensor_tensor(xp[:], xw[:], c0, cc, op0=add, op1=sub)
            # rows r0/r1/r2 of the first 21 full groups (63 blocks), then
            # the single block of the last group
            for s, e, gs, ge in ((0, 63, 0, 21), (63, 64, 21, NG)):
                n = e - s
                dst0 = P4[:, gs:ge, :, 0] if n == 63 else P4[:, gs, 0, 0:1]
                dst1 = P4[:, gs:ge, :, 1] if n == 63 else P4[:, gs, 0, 1:2]
                dst2 = P4[:, gs:ge, :, 2] if n == 63 else P4[:, gs, 0, 2:3]
                eng.scalar_tensor_tensor(
                    dst0, xp[:, s:e], float(-INV), xp[:, s:e],
                    op0=mult, op1=mult,
                )
                eng.tensor_scalar(
                    dst1, xp[:, s:e], float(2.0 * INV), None, op0=mult
                )
                eng.memset(dst2, 1.0)

        axis_basis(nc.vector, cpb[:, 0:1], coef[:, 0:1], coef[:, 2:3],
                   ccx, PX4, "x")
        axis_basis(nc.vector, cpb[:, 1:2], coef[:, 1:2], coef[:, 3:4],
                   ccy, PY4, "y")

        # PE-transpose the basis into 32-partition bands: group g of axis
        # ax covers blocks 3g..3g+2 with block bl at partition base 32*bl.
        # Matmul stationary operands must all share partition base 0 (mixing
        # PE quadrant bases breaks at runtime), so SBUF->SBUF DMAs shift the
        # 32- and 64-bands down to partition 0 afterwards.
        BT0 = {}
        with tc.tile_pool(name="ptr", bufs=2, space="PSUM") as ptr:
            ncopy = 0
            for ax, P4 in (("x", PX4), ("y", PY4)):
                BT = cpool.tile([96, NG * 128], f16, tag=f"bt{ax}")
                for g0 in range(0, NG, 4):
                    g1 = min(g0 + 4, NG)
                    tp = ptr.tile([96, 512], f16, tag="tp")
                    for g in range(g0, g1):
                        nc.tensor.transpose(
                            tp[:, (g - g0) * 128:(g - g0 + 1) * 128],
                            P4[:, g, :, :], ident,
                        )
                    w = (g1 - g0) * 128
                    src = tp[:, 0:w]
                    dst = BT[:, g0 * 128:g1 * 128]
                    if ncopy % 2 == 0:
                        nc.vector.tensor_copy(dst, src)
                    else:
                        nc.scalar.copy(dst, src)
                    ncopy += 1
                BT0[(ax, 0)] = BT
                for m in (1, 2):
                    band = cpool.tile([3, NG * 128], f16, tag=f"bt{ax}{m}")
                    nc.sync.dma_start(band[:], BT[32 * m:32 * m + 3, :])
                    BT0[(ax, m)] = band

        # journal PSUM packs two 64-aligned bands of 8 windows per tile; two
        # tiles cover all 32 pair-windows; rows 48:64 of the SBUF journal are
        # never written by the band copies (engine partition accesses must be
        # 32-aligned), so zero them once for the output DMA
        journal = cpool.tile([112, 2 * 384], f16, tag="journal")
        nc.gpsimd.memset(journal[32:64, :], 0.0)

        JP = None
        for sg in range(NSG):
            TX = pt.tile([128, BPS * WIN], f32, tag="tx")
            TY = pt.tile([128, BPS * WIN], f32, tag="ty")
            for j in range(BPS):
                b = sg * BPS + j
                g, m = b // 3, b % 3
                lx = BT0[("x", m)][0:3, g * 128:(g + 1) * 128]
                ly = BT0[("y", m)][0:3, g * 128:(g + 1) * 128]
                rb = cbs3[0:3, :]
                nc.tensor.matmul(
                    TX[:, j * WIN:(j + 1) * WIN], lhsT=lx, rhs=rb,
                    start=True, stop=True,
                )
                nc.tensor.matmul(
                    TY[:, j * WIN:(j + 1) * WIN], lhsT=ly, rhs=rb,
                    start=True, stop=True,
                )
            EX = sp.tile([128, BPS * WIN], f16, tag="ex")
            nc.scalar.activation(EX[:], TX[:], Exp)
            EY = sp.tile([128, BPS * WIN], f16, tag="ey")
            nc.scalar.activation(EY[:], TY[:], Exp)
            s = MCOL + sg * 768
            SX = sp.tile([128, BPS * WIN], f16, tag="sx")
            nc.vector.scalar_tensor_tensor(
                SX[:], EX[:], 1.0, fct[:, s:s + 384], op0=mult, op1=mult
            )
            SY = sp.tile([128, BPS * WIN], f16, tag="sy")
            nc.vector.scalar_tensor_tensor(
                SY[:], EY[:], 1.0, fct[:, s + 384:s + 768],
                op0=mult, op1=mult,
            )
            if sg % 4 == 0:
                JP = pj.tile([112, 8 * WIN], f32, tag="jp")
            for j in range(BPS):
                b = sg * BPS + j
                p = b // 2
                band, slot = (p % 16) // 8, p % 8
                nc.tensor.matmul(
                    JP[64 * band:64 * band + WIN,
                       slot * WIN:(slot + 1) * WIN],
                    lhsT=SX[:, j * WIN:(j + 1) * WIN],
                    rhs=SY[:, j * WIN:(j + 1) * WIN],
                    start=(b % 2 == 0), stop=(b % 2 == 1),
                )
            if sg % 4 == 3:
                t = sg // 4
                nc.vector.tensor_copy(
                    journal[0:48, t * 384:(t + 1) * 384], JP[0:48, :]
                )
                nc.vector.tensor_copy(
                    journal[64:112, t * 384:(t + 1) * 384], JP[64:112, :]
                )

        nc.sync.dma_start(t_out, journal[:])

    nc.compile()
    return nc


# ------------------------------------------------------------------- driver
def kernel(control_points: np.ndarray) -> np.ndarray:
    global LAST_RESULT, LAST_NC, LAST_IN_MAPS, LAST_METAS
    from concourse.bass_utils import run_bass_kernel_spmd

    cp = np.asarray(control_points, dtype=np.float32)
    in_maps, metas = _plan(cp)
    nc = _build()
    trace = bool(int(os.environ.get("BEZ_TRACE", "0")))
    try:
        res = run_bass_kernel_spmd(
            nc, in_maps, core_ids=list(range(NCORES)), trace=trace
        )
    except ModuleNotFoundError:
        res = run_bass_kernel_spmd(
            nc, in_maps, core_ids=list(range(NCORES)), trace=False
        )
    LAST_RESULT = res
    LAST_NC, LAST_IN_MAPS, LAST_METAS = nc, in_maps, metas

    out = np.zeros((RES, RES), np.float32)
    for c in range(NCORES):
        J = res.results[c]["out"].astype(np.float32)
        for p, (ox, oy) in enumerate(metas[c]):
            t, band, slot = p // 16, (p % 16) // 8, p % 8
            w = J[64 * band:64 * band + WIN,
                  t * 384 + slot * WIN: t * 384 + (slot + 1) * WIN]
            out[ox:ox + WIN, oy:oy + WIN] += w
    return out
